# revision 8
# baseline (speedup 1.0000x reference)
"""DenseCapsule (dynamic routing) kernel for 8x Trainium2 NeuronCores.

Strategy: shard in_num_caps I=4608 across the 8 cores (576 each, zero-padded
to 640 = 5 tiles of 128 partitions).  Per i-tile t the PE produces
V[o,i,k,b] = sum_d w[o,i,d,k] u[b,o,d] in o-pair PSUM tiles (uZ block trick,
one 128-wide stationary per o).  V leaves PSUM either through an Act-engine
f16 copy (then Pool multiplies by x in place) or a direct DVE multiply;
the k-reduction runs as three batched all-o tree adds into f16 logits, and
softmax for tile t overlaps the next tile's matmuls.  The weighted pass
streams y = c*x through one 128x128 w-stationary per (o,t), producing
block-diagonal partials in PSUM; diagonal blocks are gathered pair-wise by
8 strided DMAs and k-summed by a 16-column selector matmul that lands S
directly in [b, d] layout.  Two 80KB AllReduces carry the c*x_hat reduction
between iterations; the host combines the final per-core partials.
"""

import sys, os
if '/opt/trn_rl_repo' not in sys.path:
    sys.path.insert(0, '/opt/trn_rl_repo')
import numpy as np

import concourse.bass as bass
import concourse.bacc as bacc
import concourse.tile as tile
import concourse.mybir as mybir
from concourse import bass_utils

F32 = mybir.dt.float32
F16 = mybir.dt.float16
AF = mybir.ActivationFunctionType
ALU = mybir.AluOpType
AX = mybir.AxisListType

B, I, K, O, D = 128, 4608, 8, 10, 16
NCORES = 8
ISH = I // NCORES
NT = 5
IPAD = NT * 128
OD = O * D
NP = O // 2          # o-pairs per tile


def build_program(stage=4):
    nc = bacc.Bacc("TRN2", target_bir_lowering=False, debug=False,
                   num_devices=NCORES)

    xkb_d = nc.dram_tensor("xkb", [128, NT, K, B], F16, kind="ExternalInput").ap()
    w2T_d = nc.dram_tensor("w2T", [128, NT, O, K, D], F16, kind="ExternalInput").ap()
    wdT_d = nc.dram_tensor("wdT", [128, O, NT, 128], F16, kind="ExternalInput").ap()
    id_d  = nc.dram_tensor("ident", [128, 128], F32, kind="ExternalInput").ap()
    sel_d = nc.dram_tensor("sel", [128, 16], F16, kind="ExternalInput").ap()

    f01_d = nc.dram_tensor("f01", [128, OD], F32, kind="ExternalOutput").ap()
    s2_d  = nc.dram_tensor("s2b", [128, OD], F32, kind="ExternalOutput").ap()

    with tile.TileContext(nc) as tc:
        with (
            tc.tile_pool(name="big", bufs=1) as big,
            tc.tile_pool(name="stage", bufs=2) as stage_p,
            tc.tile_pool(name="work", bufs=4) as work,
            tc.tile_pool(name="wk1", bufs=3) as wk1,
            tc.tile_pool(name="small", bufs=1) as small,
            tc.tile_pool(name="psA", bufs=2, space="PSUM") as psA,
            tc.tile_pool(name="dram", bufs=2, space="DRAM") as dram,
        ):
            # ---- resident ----
            xkb  = big.tile([128, NT, K, B], F16, tag="xkb")
            w2T  = big.tile([128, NT, O, K, D], F16, tag="w2T")
            wdT  = big.tile([128, O, NT, 128], F16, tag="wdT")
            ident = big.tile([128, 128], F32, tag="ident")
            sel  = big.tile([128, 16], F16, tag="sel")
            uZ   = big.tile([128, O, K, B], F16, tag="uZ")
            L    = big.tile([128, NT, O, B], F16, tag="L")
            E    = big.tile([128, NT, O, B], F16, tag="E")
            xR   = big.tile([128, NT, K, B], F16, tag="xR")
            Sfull = big.tile([128, O, D], F32, tag="Sfull")
            u_t  = big.tile([128, O, D], F32, tag="u")
            f01  = big.tile([128, O, D], F32, tag="f01")

            def all_reduce(src_ap, shape):
                bin_ = dram.tile(shape, F32, tag="arin")
                bout = dram.tile(shape, F32, tag="arout")
                nc.sync.dma_start(bin_[:], src_ap)
                nc.gpsimd.collective_compute(
                    "AllReduce", ALU.add,
                    replica_groups=[list(range(NCORES))],
                    ins=[bin_.opt()], outs=[bout.opt()],
                )
                return bout

            nc.sync.dma_start(xkb[:], xkb_d[:])
            nc.sync.dma_start(w2T[:], w2T_d[:])
            nc.sync.dma_start(wdT[:], wdT_d[:])
            nc.sync.dma_start(ident[:], id_d[:])
            nc.sync.dma_start(sel[:], sel_d[:])
            nc.vector.memset(uZ[:], 0.0)

            def squash_into_u(S_ap, pre_scale):
                s_sc = small.tile([128, O, D], F32, tag="s_sc")
                nc.vector.tensor_scalar_mul(s_sc[:], S_ap, float(pre_scale))
                sq = small.tile([128, O, D], F32, tag="sq")
                nc.vector.tensor_mul(sq[:], s_sc[:], s_sc[:])
                n2 = small.tile([128, O], F32, tag="n2")
                nc.vector.reduce_sum(n2[:], sq[:], axis=AX.X)
                n1 = small.tile([128, O], F32, tag="n1")
                nc.scalar.activation(n1[:], n2[:], AF.Ln)
                nc.scalar.activation(n1[:], n1[:], AF.Exp, scale=0.5)
                den = small.tile([128, O], F32, tag="den")
                nc.vector.tensor_scalar_add(den[:], n2[:], 1.0)
                rden = small.tile([128, O], F32, tag="rden")
                nc.vector.reciprocal(rden[:], den[:])
                sig = small.tile([128, O], F32, tag="sig")
                nc.vector.tensor_mul(sig[:], n1[:], rden[:])
                sig_b = sig[:].unsqueeze(2).broadcast_to([128, O, D])
                nc.vector.tensor_mul(u_t[:], s_sc[:], sig_b)

            def build_uZ():
                """uZ[16k+d, o, k, b] = u_t[b, o, d]; other slots stay 0."""
                for o in range(O):
                    pt = psA.tile([128, 2, K, B], F32, tag="ps")
                    ptv = pt[0:16, 0, 0, :]
                    nc.tensor.matmul(ptv, u_t[:, o, :], ident[:], is_transpose=True)
                    nc.scalar.copy(uZ[0:16, o, 0, :], ptv)
                for k in range(1, K):
                    nc.sync.dma_start(uZ[16*k:16*k+16, :, k, :], uZ[0:16, :, 0, :])

            def logit_and_softmax(first):
                for t in range(NT):
                    P5 = stage_p.tile([128, O, K, B], F16, tag="P5")
                    for op in range(NP):
                        o0 = 2 * op
                        psV = psA.tile([128, 2, K, B], F32, tag="ps")
                        for j in (0, 1):
                            o = o0 + j
                            nc.tensor.matmul(psV[:, j, 0:4, :], wdT[:, o, t, :], uZ[:, o, 0:4, :])
                            nc.tensor.matmul(psV[:, j, 4:8, :], wdT[:, o, t, :], uZ[:, o, 4:8, :])
                        xb2 = xkb[:, t].unsqueeze(1).broadcast_to([128, 2, K, B])
                        if op % 2 == 1:
                            # DVE multiplies straight out of PSUM
                            nc.vector.tensor_mul(P5[:, o0:o0+2], psV[:], xb2)
                        else:
                            # Act egress to f16, Pool multiplies in place
                            nc.scalar.copy(P5[:, o0:o0+2], psV[:])
                            nc.gpsimd.tensor_mul(P5[:, o0:o0+2], P5[:, o0:o0+2], xb2)
                    # batched k-reduction trees over all 10 o's
                    nc.gpsimd.tensor_add(P5[:, :, 0:2], P5[:, :, 0:2], P5[:, :, 4:6])
                    nc.vector.tensor_add(P5[:, :, 2:4], P5[:, :, 2:4], P5[:, :, 6:8])
                    nc.vector.tensor_add(P5[:, :, 0:2], P5[:, :, 0:2], P5[:, :, 2:4])
                    if first:
                        nc.vector.tensor_add(L[:, t], P5[:, :, 0, :], P5[:, :, 1, :])
                    else:
                        Lt = wk1.tile([128, O, B], F16, tag="Lt")
                        nc.gpsimd.tensor_add(Lt[:], P5[:, :, 0, :], P5[:, :, 1, :])
                        nc.vector.tensor_add(L[:, t], L[:, t], Lt[:])
                    # softmax for tile t (overlaps next tile's matmuls)
                    nc.scalar.activation(E[:, t], L[:, t], AF.Exp)
                    A5 = wk1.tile([128, 5, B], F16, tag="A5")
                    nc.gpsimd.tensor_add(A5[:], E[:, t, 0:5], E[:, t, 5:10])
                    nc.vector.tensor_add(A5[:, 0:2], A5[:, 0:2], A5[:, 2:4])
                    Z = wk1.tile([128, B], F16, tag="Z")
                    nc.vector.tensor_add(Z[:], A5[:, 0], A5[:, 1])
                    nc.vector.tensor_add(Z[:], Z[:], A5[:, 4])
                    R = wk1.tile([128, B], F32, tag="R")
                    nc.vector.reciprocal(R[:], Z[:])
                    R_b = R[:].unsqueeze(1).broadcast_to([128, K, B])
                    nc.gpsimd.tensor_mul(xR[:, t], xkb[:, t], R_b)

            def weighted_pass():
                for op in range(NP):
                    o0 = 2 * op
                    psW = psA.tile([128, 2, K, B], F32, tag="ps")
                    for t in range(NT):
                        y2 = work.tile([128, 2, K, B], F16, tag="y2")
                        x_b = xR[:, t].unsqueeze(1).broadcast_to([128, 2, K, B])
                        E_b = E[:, t, o0:o0+2, :].unsqueeze(2).broadcast_to([128, 2, K, B])
                        ye = nc.vector if op % 2 == 0 else nc.gpsimd
                        ye.tensor_mul(y2[:], x_b, E_b)
                        for j in (0, 1):
                            stat = w2T[:, t, o0 + j].rearrange("p k d -> p (k d)")
                            nc.tensor.matmul(psW[:, j, 0:4, :], stat, y2[:, j, 0:4, :],
                                             start=(t == 0), stop=(t == NT - 1))
                            nc.tensor.matmul(psW[:, j, 4:8, :], stat, y2[:, j, 4:8, :],
                                             start=(t == 0), stop=(t == NT - 1))
                    W16 = work.tile([128, 2, K, B], F16, tag="W16")
                    nc.scalar.copy(W16[:], psW[:])
                    Do2 = wk1.tile([128, 2, B], F16, tag="Do2")
                    for k in range(K):
                        nc.sync.dma_start(Do2[16*k:16*k+16, :, :], W16[16*k:16*k+16, :, k, :])
                    psF = psA.tile([128, 2, K, B], F32, tag="ps")
                    for j in (0, 1):
                        nc.tensor.matmul(psF[:, j, 0, 0:16], Do2[:, j, :], sel[:])
                    nc.scalar.copy(Sfull[:, o0:o0+2, :], psF[:, 0:2, 0, 0:16])

            # ================= iteration 0 =================
            ps0 = psA.tile([128, 2, K, B], F32, tag="ps")
            ps0v = ps0[:].rearrange("p j k b -> p (j k b)")[:, 0:OD]
            for t in range(NT):
                for k in range(K):
                    nc.tensor.matmul(
                        ps0v,
                        xkb[:, t, k, :],
                        w2T[:, t, :, k, :],
                        start=(t == 0 and k == 0),
                        stop=(t == NT - 1 and k == K - 1))
            sp0 = small.tile([128, O, D], F32, tag="sp0")
            nc.scalar.copy(sp0[:].rearrange("p o d -> p (o d)"), ps0v)
            bout0 = all_reduce(sp0[:].rearrange("p o d -> p (o d)"), [128, OD])
            nc.sync.dma_start(Sfull[:].rearrange("p o d -> p (o d)"), bout0[:])
            squash_into_u(Sfull[:], 0.1)
            nc.vector.tensor_scalar_mul(f01[:], u_t[:], 0.3)
            build_uZ()

            # ================= iteration 1 =================
            logit_and_softmax(first=True)
            weighted_pass()
            bout1 = all_reduce(Sfull[:].rearrange("p o d -> p (o d)"), [128, OD])
            nc.sync.dma_start(Sfull[:].rearrange("p o d -> p (o d)"), bout1[:])
            squash_into_u(Sfull[:], 1.0)
            nc.vector.scalar_tensor_tensor(
                f01[:], u_t[:], 0.3, f01[:], op0=ALU.mult, op1=ALU.add)
            build_uZ()

            # ================= iteration 2 =================
            logit_and_softmax(first=False)
            weighted_pass()

            nc.sync.dma_start(f01_d[:], f01[:].rearrange("p o d -> p (o d)"))
            nc.sync.dma_start(s2_d[:], Sfull[:].rearrange("p o d -> p (o d)"))

    nc.compile()
    return nc


def prep_core_inputs(x, w, core):
    xs = x[:, core * ISH:(core + 1) * ISH, :].astype(np.float32)
    ws = w[:, core * ISH:(core + 1) * ISH].astype(np.float32)
    xsp = np.zeros((B, IPAD, K), np.float32); xsp[:, :ISH] = xs
    wsp = np.zeros((O, IPAD, D, K), np.float32); wsp[:, :ISH] = ws

    xT = xsp.reshape(B, NT, 128, K).transpose(2, 1, 3, 0).copy()       # [p,t,k,b]
    w5 = wsp.reshape(O, NT, 128, D, K)
    w2T = w5.transpose(2, 1, 0, 4, 3).copy()                           # [p,t,o,k,d]
    wdT = np.zeros((128, O, NT, 128), np.float32)
    for k in range(K):
        wdT[16 * k:16 * k + 16] = w5[:, :, :, :, k].transpose(3, 0, 1, 2)
    sel = np.zeros((128, 16), np.float32)
    for k in range(K):
        sel[16 * k:16 * k + 16] = np.eye(16, dtype=np.float32)
    return {
        "xkb": xT.astype(np.float16),
        "w2T": w2T.astype(np.float16),
        "wdT": wdT.astype(np.float16),
        "ident": np.eye(128, dtype=np.float32),
        "sel": sel.astype(np.float16),
    }


def host_epilogue(f01_core0, s2_list):
    s2 = np.sum(np.stack(s2_list), axis=0, dtype=np.float32).reshape(B, O, D)
    n = np.linalg.norm(s2, axis=-1, keepdims=True).astype(np.float32)
    scale = (n ** 2 / (1.0 + n ** 2) / (n + 1e-8)).astype(np.float32)
    u2 = (scale * s2).astype(np.float32)
    out = f01_core0.reshape(B, O, D).astype(np.float32) + np.float32(0.4) * u2
    return out.reshape(B, O, 4, 4).astype(np.float32)


def run(x, w, nc=None, trace=False, tmpdir=None):
    if nc is None:
        nc = build_program()
    in_maps = [prep_core_inputs(x, w, c) for c in range(NCORES)]
    res = bass_utils.run_bass_kernel_spmd(
        nc, in_maps, core_ids=list(range(NCORES)), trace=trace, tmpdir=tmpdir)
    out = host_epilogue(res.results[0]["f01"],
                        [res.results[c]["s2b"] for c in range(NCORES)])
    return out, res


_NC_CACHE = {}

def _get_program():
    if "nc" not in _NC_CACHE:
        _NC_CACHE["nc"] = build_program()
    return _NC_CACHE["nc"]


def kernel(x, weight):
    x = np.asarray(x, dtype=np.float32)
    w = np.asarray(weight, dtype=np.float32)
    out, _ = run(x, w, nc=_get_program())
    return out


# revision 9
# speedup vs baseline: 1.3186x; 1.3186x over previous
"""DenseCapsule (dynamic routing) kernel for 8x Trainium2 NeuronCores.

Strategy: shard in_num_caps I=4608 across the 8 cores (576 each, zero-padded
to 640 = 5 tiles of 128 partitions).  Per i-tile t the PE produces
V[o,i,k,b] = sum_d w[o,i,d,k] u[b,o,d] in o-pair PSUM tiles (uZ block trick,
one 128-wide stationary per o).  V leaves PSUM either through an Act-engine
f16 copy (then Pool multiplies by x in place) or a direct DVE multiply;
the k-reduction runs as three batched all-o tree adds into f16 logits, and
softmax for tile t overlaps the next tile's matmuls.  The weighted pass
streams y = c*x through one 128x128 w-stationary per (o,t), producing
block-diagonal partials in PSUM; diagonal blocks are gathered pair-wise by
8 strided DMAs and k-summed by a 16-column selector matmul that lands S
directly in [b, d] layout.  Two 80KB AllReduces carry the c*x_hat reduction
between iterations; the host combines the final per-core partials.
"""

import sys, os
if '/opt/trn_rl_repo' not in sys.path:
    sys.path.insert(0, '/opt/trn_rl_repo')
import numpy as np

import concourse.bass as bass
import concourse.bacc as bacc
import concourse.tile as tile
import concourse.mybir as mybir
from concourse import bass_utils

F32 = mybir.dt.float32
F16 = mybir.dt.float16
AF = mybir.ActivationFunctionType
ALU = mybir.AluOpType
AX = mybir.AxisListType

B, I, K, O, D = 128, 4608, 8, 10, 16
NCORES = 8
ISH = I // NCORES
NT = 5
IPAD = NT * 128
OD = O * D
NP = O // 2          # o-pairs per tile


def build_program(stage=4):
    nc = bacc.Bacc("TRN2", target_bir_lowering=False, debug=False,
                   num_devices=NCORES)

    xkb_d = nc.dram_tensor("xkb", [128, NT, K, B], F16, kind="ExternalInput").ap()
    w2T_d = nc.dram_tensor("w2T", [128, NT, O, K, D], F16, kind="ExternalInput").ap()
    wdT_d = nc.dram_tensor("wdT", [128, O, NT, 128], F16, kind="ExternalInput").ap()
    id_d  = nc.dram_tensor("ident", [128, 128], F32, kind="ExternalInput").ap()
    sel_d = nc.dram_tensor("sel", [128, 16], F16, kind="ExternalInput").ap()

    f01_d = nc.dram_tensor("f01", [128, OD], F32, kind="ExternalOutput").ap()
    s2_d  = nc.dram_tensor("s2b", [128, OD], F32, kind="ExternalOutput").ap()

    with tile.TileContext(nc) as tc:
        with (
            tc.tile_pool(name="big", bufs=1) as big,
            tc.tile_pool(name="stage", bufs=2) as stage_p,
            tc.tile_pool(name="work", bufs=4) as work,
            tc.tile_pool(name="wk1", bufs=3) as wk1,
            tc.tile_pool(name="small", bufs=1) as small,
            tc.tile_pool(name="psA", bufs=2, space="PSUM") as psA,
            tc.tile_pool(name="dram", bufs=2, space="DRAM") as dram,
        ):
            # ---- resident ----
            xkb  = big.tile([128, NT, K, B], F16, tag="xkb")
            w2T  = big.tile([128, NT, O, K, D], F16, tag="w2T")
            wdT  = big.tile([128, O, NT, 128], F16, tag="wdT")
            ident = big.tile([128, 128], F32, tag="ident")
            sel  = big.tile([128, 16], F16, tag="sel")
            uZ   = big.tile([128, O, K, B], F16, tag="uZ")
            L    = big.tile([128, NT, O, B], F16, tag="L")
            E    = big.tile([128, NT, O, B], F16, tag="E")
            xR   = big.tile([128, NT, K, B], F16, tag="xR")
            Sfull = big.tile([128, O, D], F32, tag="Sfull")
            u_t  = big.tile([128, O, D], F32, tag="u")
            f01  = big.tile([128, O, D], F32, tag="f01")

            def all_reduce(src_ap, shape):
                bin_ = dram.tile(shape, F32, tag="arin")
                bout = dram.tile(shape, F32, tag="arout")
                nc.sync.dma_start(bin_[:], src_ap)
                nc.gpsimd.collective_compute(
                    "AllReduce", ALU.add,
                    replica_groups=[list(range(NCORES))],
                    ins=[bin_.opt()], outs=[bout.opt()],
                )
                return bout

            nc.sync.dma_start(xkb[:], xkb_d[:])
            nc.sync.dma_start(w2T[:], w2T_d[:])
            nc.sync.dma_start(wdT[:], wdT_d[:])
            nc.sync.dma_start(ident[:], id_d[:])
            nc.sync.dma_start(sel[:], sel_d[:])
            nc.vector.memset(uZ[:], 0.0)

            def squash_into_u(S_ap, pre_scale):
                s_sc = small.tile([128, O, D], F32, tag="s_sc")
                nc.vector.tensor_scalar_mul(s_sc[:], S_ap, float(pre_scale))
                sq = small.tile([128, O, D], F32, tag="sq")
                nc.vector.tensor_mul(sq[:], s_sc[:], s_sc[:])
                n2 = small.tile([128, O], F32, tag="n2")
                nc.vector.reduce_sum(n2[:], sq[:], axis=AX.X)
                n1 = small.tile([128, O], F32, tag="n1")
                nc.scalar.activation(n1[:], n2[:], AF.Ln)
                nc.scalar.activation(n1[:], n1[:], AF.Exp, scale=0.5)
                den = small.tile([128, O], F32, tag="den")
                nc.vector.tensor_scalar_add(den[:], n2[:], 1.0)
                rden = small.tile([128, O], F32, tag="rden")
                nc.vector.reciprocal(rden[:], den[:])
                sig = small.tile([128, O], F32, tag="sig")
                nc.vector.tensor_mul(sig[:], n1[:], rden[:])
                sig_b = sig[:].unsqueeze(2).broadcast_to([128, O, D])
                nc.vector.tensor_mul(u_t[:], s_sc[:], sig_b)

            def build_uZ():
                """uZ[16k+d, o, k, b] = u_t[b, o, d]; other slots stay 0."""
                for o in range(O):
                    pt = psA.tile([128, 2, K, B], F32, tag="ps")
                    ptv = pt[0:16, 0, 0, :]
                    nc.tensor.matmul(ptv, u_t[:, o, :], ident[:], is_transpose=True)
                    nc.scalar.copy(uZ[0:16, o, 0, :], ptv)
                for k in range(1, K):
                    nc.sync.dma_start(uZ[16*k:16*k+16, :, k, :], uZ[0:16, :, 0, :])

            # tile ownership: DVE-owned tiles multiply straight from PSUM
            # and tree-reduce on DVE; Pool-owned tiles take an Act f16 copy
            # then multiply+reduce on Pool.  All slices are flat/contiguous.
            DVE_OWNED = (0, 2, 4, 6, 8, 9)

            def logit_and_softmax(first):
                for t in range(NT):
                    for o in range(O):
                        psV = psA.tile([128, 2, K, B], F32, tag="ps")
                        nc.tensor.matmul(psV[:, 0, 0:4, :], wdT[:, o, t, :], uZ[:, o, 0:4, :])
                        nc.tensor.matmul(psV[:, 0, 4:8, :], wdT[:, o, t, :], uZ[:, o, 4:8, :])
                        P = work.tile([128, K, B], F16, tag="P")
                        if o in DVE_OWNED:
                            eng = nc.vector
                            eng.tensor_mul(P[:], psV[:, 0], xkb[:, t])
                        else:
                            eng = nc.gpsimd
                            nc.scalar.copy(P[:], psV[:, 0])
                            eng.tensor_mul(P[:], P[:], xkb[:, t])
                        eng.tensor_add(P[:, 0:4], P[:, 0:4], P[:, 4:8])
                        eng.tensor_add(P[:, 0:2], P[:, 0:2], P[:, 2:4])
                        if first:
                            eng.tensor_add(L[:, t, o, :], P[:, 0], P[:, 1])
                        else:
                            eng.tensor_add(P[:, 0], P[:, 0], P[:, 1])
                            eng.tensor_add(L[:, t, o, :], L[:, t, o, :], P[:, 0])
                    # softmax for tile t (overlaps next tile's matmuls)
                    nc.scalar.activation(E[:, t], L[:, t], AF.Exp)
                    A5 = wk1.tile([128, 5, B], F16, tag="A5")
                    nc.vector.tensor_add(A5[:], E[:, t, 0:5], E[:, t, 5:10])
                    nc.vector.tensor_add(A5[:, 0:2], A5[:, 0:2], A5[:, 2:4])
                    Z = wk1.tile([128, B], F16, tag="Z")
                    nc.vector.tensor_add(Z[:], A5[:, 0], A5[:, 1])
                    nc.vector.tensor_add(Z[:], Z[:], A5[:, 4])
                    R = wk1.tile([128, B], F32, tag="R")
                    nc.vector.reciprocal(R[:], Z[:])
                    R_b = R[:].unsqueeze(1).broadcast_to([128, K, B])
                    nc.gpsimd.tensor_mul(xR[:, t], xkb[:, t], R_b)

            def weighted_pass():
                for op in range(NP):
                    o0 = 2 * op
                    psW = psA.tile([128, 2, K, B], F32, tag="ps")
                    for t in range(NT):
                        for j in (0, 1):
                            o = o0 + j
                            y = work.tile([128, K, B], F16, tag="y")
                            E_b = E[:, t, o, :].unsqueeze(1).broadcast_to([128, K, B])
                            ye = nc.vector if o in DVE_OWNED else nc.gpsimd
                            ye.tensor_mul(y[:], xR[:, t], E_b)
                            stat = w2T[:, t, o].rearrange("p k d -> p (k d)")
                            nc.tensor.matmul(psW[:, j, 0:4, :], stat, y[:, 0:4, :],
                                             start=(t == 0), stop=(t == NT - 1))
                            nc.tensor.matmul(psW[:, j, 4:8, :], stat, y[:, 4:8, :],
                                             start=(t == 0), stop=(t == NT - 1))
                    W16 = work.tile([128, 2, K, B], F16, tag="W16")
                    nc.scalar.copy(W16[:], psW[:])
                    Do2 = wk1.tile([128, 2, B], F16, tag="Do2")
                    for k in range(K):
                        nc.sync.dma_start(Do2[16*k:16*k+16, :, :], W16[16*k:16*k+16, :, k, :])
                    psF = psA.tile([128, 2, K, B], F32, tag="ps")
                    for j in (0, 1):
                        nc.tensor.matmul(psF[:, j, 0, 0:16], Do2[:, j, :], sel[:])
                    nc.scalar.copy(Sfull[:, o0:o0+2, :], psF[:, 0:2, 0, 0:16])

            # ================= iteration 0 =================
            ps0 = psA.tile([128, 2, K, B], F32, tag="ps")
            ps0v = ps0[:].rearrange("p j k b -> p (j k b)")[:, 0:OD]
            for t in range(NT):
                for k in range(K):
                    nc.tensor.matmul(
                        ps0v,
                        xkb[:, t, k, :],
                        w2T[:, t, :, k, :],
                        start=(t == 0 and k == 0),
                        stop=(t == NT - 1 and k == K - 1))
            sp0 = small.tile([128, O, D], F32, tag="sp0")
            nc.scalar.copy(sp0[:].rearrange("p o d -> p (o d)"), ps0v)
            bout0 = all_reduce(sp0[:].rearrange("p o d -> p (o d)"), [128, OD])
            nc.sync.dma_start(Sfull[:].rearrange("p o d -> p (o d)"), bout0[:])
            squash_into_u(Sfull[:], 0.1)
            nc.vector.tensor_scalar_mul(f01[:], u_t[:], 0.3)
            build_uZ()

            # ================= iteration 1 =================
            logit_and_softmax(first=True)
            weighted_pass()
            bout1 = all_reduce(Sfull[:].rearrange("p o d -> p (o d)"), [128, OD])
            nc.sync.dma_start(Sfull[:].rearrange("p o d -> p (o d)"), bout1[:])
            squash_into_u(Sfull[:], 1.0)
            nc.vector.scalar_tensor_tensor(
                f01[:], u_t[:], 0.3, f01[:], op0=ALU.mult, op1=ALU.add)
            build_uZ()

            # ================= iteration 2 =================
            logit_and_softmax(first=False)
            weighted_pass()

            nc.sync.dma_start(f01_d[:], f01[:].rearrange("p o d -> p (o d)"))
            nc.sync.dma_start(s2_d[:], Sfull[:].rearrange("p o d -> p (o d)"))

    nc.compile()
    return nc


def prep_core_inputs(x, w, core):
    xs = x[:, core * ISH:(core + 1) * ISH, :].astype(np.float32)
    ws = w[:, core * ISH:(core + 1) * ISH].astype(np.float32)
    xsp = np.zeros((B, IPAD, K), np.float32); xsp[:, :ISH] = xs
    wsp = np.zeros((O, IPAD, D, K), np.float32); wsp[:, :ISH] = ws

    xT = xsp.reshape(B, NT, 128, K).transpose(2, 1, 3, 0).copy()       # [p,t,k,b]
    w5 = wsp.reshape(O, NT, 128, D, K)
    w2T = w5.transpose(2, 1, 0, 4, 3).copy()                           # [p,t,o,k,d]
    wdT = np.zeros((128, O, NT, 128), np.float32)
    for k in range(K):
        wdT[16 * k:16 * k + 16] = w5[:, :, :, :, k].transpose(3, 0, 1, 2)
    sel = np.zeros((128, 16), np.float32)
    for k in range(K):
        sel[16 * k:16 * k + 16] = np.eye(16, dtype=np.float32)
    return {
        "xkb": xT.astype(np.float16),
        "w2T": w2T.astype(np.float16),
        "wdT": wdT.astype(np.float16),
        "ident": np.eye(128, dtype=np.float32),
        "sel": sel.astype(np.float16),
    }


def host_epilogue(f01_core0, s2_list):
    s2 = np.sum(np.stack(s2_list), axis=0, dtype=np.float32).reshape(B, O, D)
    n = np.linalg.norm(s2, axis=-1, keepdims=True).astype(np.float32)
    scale = (n ** 2 / (1.0 + n ** 2) / (n + 1e-8)).astype(np.float32)
    u2 = (scale * s2).astype(np.float32)
    out = f01_core0.reshape(B, O, D).astype(np.float32) + np.float32(0.4) * u2
    return out.reshape(B, O, 4, 4).astype(np.float32)


def run(x, w, nc=None, trace=False, tmpdir=None):
    if nc is None:
        nc = build_program()
    in_maps = [prep_core_inputs(x, w, c) for c in range(NCORES)]
    res = bass_utils.run_bass_kernel_spmd(
        nc, in_maps, core_ids=list(range(NCORES)), trace=trace, tmpdir=tmpdir)
    out = host_epilogue(res.results[0]["f01"],
                        [res.results[c]["s2b"] for c in range(NCORES)])
    return out, res


_NC_CACHE = {}

def _get_program():
    if "nc" not in _NC_CACHE:
        _NC_CACHE["nc"] = build_program()
    return _NC_CACHE["nc"]


def kernel(x, weight):
    x = np.asarray(x, dtype=np.float32)
    w = np.asarray(weight, dtype=np.float32)
    out, _ = run(x, w, nc=_get_program())
    return out
